# revision 33
# baseline (speedup 1.0000x reference)
"""Trainium2 Bass kernel for a 2-layer GAT-style reduction network.

Reference math (per head h, per group of 16 nodes):
    wx   = x @ W[h]                                  # [*, n, d]
    z    = gelu(wx @ A_top[h] + wx[root] @ A_bot[h]) # root = node 0 of group
    att  = softmax(gelu(z), over n)
    out_h[g] = gelu(sum_n att[n] * wx[n])
    layer out = mean_h out_h
Applied twice: layer0 groups = N1(16) within (b, n2); layer1 groups = N2(16)
within b.

Algebraic restructure:
  - sum_n att[n]*(x[n] @ W) == (sum_n att[n]*x[n]) @ W, so the big matmul
    runs per-group (1024 rows/core) instead of per-node (16384 rows/core).
  - wx @ A_top == x @ (W @ A_top) := x @ a_eff; scores come from thin
    [F, 4] matmuls (a_eff per head), b-scores only at root tokens.
  - head-mean of layer0 absorbed into layer1 weights (0.25 * W1/aeff1);
    final 0.25 applied explicitly.

Distribution: pure data-parallel over 8 NeuronCores (batch 512 -> 64/core).
Weights replicated, bf16. No collectives. x cast to bf16 on the host.

Key structural points vs a naive pipeline (all chosen from measurement:
per-execution overhead is ~212us + ~24us/MB of input bytes, so inputs are
kept minimal; SWDGE SBUF<->SBUF packing DMAs cost ~2.6us each and are
eliminated entirely):
  - a/b attention scores accumulate DIRECTLY in PSUM across all 8 blocks of
    a half-layer via zero-padded stationary weights (aebA/aebB variants with
    the 4 head-columns placed at 4*(b%8)); no score-packing DMAs at all.
    Row layout of the [64, 512] score tile: 32*hf + 4*(b%8) + h, columns =
    tokens within the hf half-block; b-scores ([64, 32]) hold root tokens
    only (one matmul per (block, fs) with a stride-16 moving AP).
  - softmax epilogue runs batched per 8-block half ([64, 512] tiles, full
    ACT lanes), gelu/exp table switches batched (~6 per rep).
  - att transpose for stage-1: ONE PE transpose per 128-token window of the
    [64, 512] att tile ([64, 128] -> [128, 64]); a single DVE mul per block
    builds the masked S_att [128, 8, 32] from the transposed columns.
  - x transposed on-chip per block (PE transpose, 8 per fs into one PSUM
    bank, one [128, 1024] bf16 drain per fs split across DVE/ACT).
  - stage-2 weights + ybuf in bf16 (FWL weight loads, halved drain cost).
  - layer-1 scores use the same zero-padded-accumulation trick per 256-token
    quarter ([16, 256] score tile, rows 4*qt + h).
  - stage-1 accumulation banks alternate between ps_s1 and the (then-idle)
    ps_big pool for the first two chunks to avoid drain-latency slot stalls;
    per-chunk S_att builds are hoisted ahead of the matmuls; one final
    output DMA (each DMA costs ~2us completion latency); layer-1 score
    matmuls run as one long accumulation group overlapped under the E
    phase, and the x1n transposes batch 4-per-PSUM-bank with one drain
    each, overlapping the layer-1 epilogue chain.

Avoid: SBUF->SBUF transpose=True DMAs (~300us each on HW) and
partition-crossing multi-dim rearranges in DMA APs (silently scramble data).
"""

import sys

sys.path.insert(0, "/opt/trn_rl_repo")

import numpy as np
from contextlib import ExitStack

import concourse.bass as bass
import concourse.tile as tile
from concourse import bacc, mybir
from concourse.bass_utils import run_bass_kernel_spmd

dt = mybir.dt
AF = mybir.ActivationFunctionType

NCORES = 8
B, N2, N1, F, D, H = 512, 16, 16, 256, 256, 4
BS = B // NCORES  # 64 samples per core
T0 = BS * N2 * N1  # 16384 tokens, layer 0
NB0 = T0 // 1024  # 16 blocks of 1024 tokens
T1 = BS * N2  # 1024 tokens, layer 1

import os as _os

REPS = int(_os.environ.get("KREPS", "1"))
TRACE = False
_CACHE = {}


def f32(ap):
    return ap.bitcast(dt.float32)


def build_program():
    nc = bacc.Bacc("TRN2", target_bir_lowering=False, debug=False)

    x_d = nc.dram_tensor("x", [T0, F], dt.bfloat16, kind="ExternalInput").ap()
    w0_d = nc.dram_tensor("w0", [H, F, D], dt.bfloat16, kind="ExternalInput").ap()
    w1_d = nc.dram_tensor("w1", [H, D, D], dt.bfloat16, kind="ExternalInput").ap()
    # compact score weights: [fs, p, 0:4]=a_eff heads, [fs, p, 4:8]=b_eff
    aeb_d = nc.dram_tensor("aeb", [2, 128, 8], dt.bfloat16, kind="ExternalInput").ap()
    ae1_d = nc.dram_tensor("ae1", [2, 128, 8], dt.bfloat16, kind="ExternalInput").ap()
    mm_d = nc.dram_tensor("mmask", [128, 32], dt.float32, kind="ExternalInput").ap()
    id128_d = nc.dram_tensor("id128", [128, 128], dt.float32r, kind="ExternalInput").ap()
    id128b_d = nc.dram_tensor("id128b", [128, 128], dt.bfloat16, kind="ExternalInput").ap()
    out_d = nc.dram_tensor("out", [BS, D], dt.float32, kind="ExternalOutput").ap()

    with tile.TileContext(nc) as tc, ExitStack() as ctx:
        cpool = ctx.enter_context(tc.tile_pool(name="consts", bufs=1))
        xbpool = ctx.enter_context(tc.tile_pool(name="xb", bufs=8))
        xtpool = ctx.enter_context(tc.tile_pool(name="xt", bufs=9))
        attpool = ctx.enter_context(tc.tile_pool(name="att", bufs=2))
        epool = ctx.enter_context(tc.tile_pool(name="eps", bufs=2))
        sapool = ctx.enter_context(tc.tile_pool(name="sab", bufs=5))
        ybpool = ctx.enter_context(tc.tile_pool(name="ybuf", bufs=2))
        ghpool = ctx.enter_context(tc.tile_pool(name="gh", bufs=4))
        adpool = ctx.enter_context(tc.tile_pool(name="ad", bufs=2))
        x1pool = ctx.enter_context(tc.tile_pool(name="x1", bufs=1))
        mpool = ctx.enter_context(tc.tile_pool(name="misc", bufs=1))

        # PSUM: 4 pools x 2 slots x 1 bank = 8 banks exactly.
        #   ps_big: x-transpose staging banks (B) -> stage-2 o2 (E) -> L1 o21/otp
        #   ps_z:   zaps_q [64, 512] (B/C) -> L1 z1a
        #   ps_r:   zbr_q [64, 32] (B/C) -> attT atp_q (D) -> L1 zbr1/atp1
        #   ps_s1:  stage-1 ybps -> L1 y1p
        ps_big = ctx.enter_context(tc.tile_pool(name="ps_big", bufs=2, space="PSUM"))
        ps_z = ctx.enter_context(tc.tile_pool(name="ps_z", bufs=2, space="PSUM"))
        ps_r = ctx.enter_context(tc.tile_pool(name="ps_r", bufs=2, space="PSUM"))
        ps_s1 = ctx.enter_context(tc.tile_pool(name="ps_s1", bufs=2, space="PSUM"))

        # ---- constants (scalar queue; x loads own the sync queue) ----
        w0_t = cpool.tile([128, H, 2, D], dt.bfloat16, tag="w0")
        w1_t = cpool.tile([128, H, 2, D], dt.bfloat16, tag="w1")
        aeb_t = cpool.tile([128, 2, 8], dt.bfloat16, tag="aeb")
        nc.scalar.dma_start(out=aeb_t[:], in_=aeb_d.rearrange("s p j -> p s j"))
        ae1c_t = cpool.tile([128, 2, 8], dt.bfloat16, tag="ae1c")
        nc.scalar.dma_start(out=ae1c_t[:], in_=ae1_d.rearrange("s p j -> p s j"))
        # build zero-padded per-block-variant stationaries on-device
        aebA_t = cpool.tile([128, 8, 2, 32], dt.bfloat16, tag="aebA")
        aebB_t = cpool.tile([128, 8, 2, 32], dt.bfloat16, tag="aebB")
        nc.gpsimd.memset(aebA_t[:], 0.0)
        nc.gpsimd.memset(aebB_t[:], 0.0)
        for b8 in range(8):
            nc.vector.tensor_copy(
                aebA_t[:, b8, :, 4 * b8 : 4 * b8 + 4], aeb_t[:, :, 0:4]
            )
            nc.vector.tensor_copy(
                aebB_t[:, b8, :, 4 * b8 : 4 * b8 + 4], aeb_t[:, :, 4:8]
            )
        ae1A_t = cpool.tile([128, 4, 2, 16], dt.bfloat16, tag="ae1A")
        ae1B_t = cpool.tile([128, 4, 2, 16], dt.bfloat16, tag="ae1B")
        nc.gpsimd.memset(ae1A_t[:], 0.0)
        nc.gpsimd.memset(ae1B_t[:], 0.0)
        for qt in range(4):
            nc.vector.tensor_copy(
                ae1A_t[:, qt, :, 4 * qt : 4 * qt + 4], ae1c_t[:, :, 0:4]
            )
            nc.vector.tensor_copy(
                ae1B_t[:, qt, :, 4 * qt : 4 * qt + 4], ae1c_t[:, :, 4:8]
            )
        mm_t = cpool.tile([128, 32], dt.float32, tag="mm")
        nc.scalar.dma_start(out=mm_t[:], in_=mm_d)
        id128_t = cpool.tile([128, 128], dt.float32r, tag="id128")
        nc.scalar.dma_start(out=id128_t[:], in_=id128_d)
        id128b_t = cpool.tile([128, 128], dt.bfloat16, tag="id128b")
        nc.scalar.dma_start(out=id128b_t[:], in_=id128b_d)

        for rep in range(REPS):
            # ============ LAYER 0 ============
            x1T = x1pool.tile([128, 2, 1024], dt.bfloat16, tag="x1T", name="x1T")
            x1n = x1pool.tile([128, 8, 256], dt.bfloat16, tag="x1n", name="x1n")

            xbs = {}  # pair index -> [128, 16, 256] bf16 (2 blocks)
            xtbs = {}  # block -> [128, 2, 1024] bf16 transposed x
            zaps = {}  # q -> [64, 512] psum, rows 32*hf + 4*b8 + h
            zbrp = {}  # q -> [64, 32] psum, root b-scores per group
            att_sb = {}  # q -> [64, 512] f32r att weights
            atp = {}  # q -> [128, 4, 64] f32r transposed att (psum)
            ndr = [0]

            def emit_xload(pp):
                xb = xbpool.tile([128, 16, 256], dt.bfloat16, tag="xb", name="xb")
                if pp == 0:
                    # quarter-split the first load so block-0 transposes start
                    # as soon as the first 4 token-slices land
                    for qtr in range(4):
                        nc.sync.dma_start(
                            out=xb[:, 4 * qtr : 4 * (qtr + 1), :],
                            in_=x_d[512 * qtr : 512 * (qtr + 1), :].rearrange(
                                "(n p) f -> p n f", p=128
                            ),
                        )
                else:
                    nc.sync.dma_start(
                        out=xb[:],
                        in_=x_d[2048 * pp : 2048 * (pp + 1), :].rearrange(
                            "(n p) f -> p n f", p=128
                        ),
                    )
                xbs[pp] = xb

            def emit_Bt(b):
                # PE-transpose block b: 8 [128,128] transposes per fs into one
                # PSUM bank, one [128, 1024] bf16 drain per fs (DVE/ACT split)
                xb = xbs[b // 2]
                ko = 8 * (b % 2)
                xtb = xtpool.tile([128, 2, 1024], dt.bfloat16, tag="xt", name="xtb")
                for fs in range(2):
                    tp = ps_big.tile([128, 1024], dt.bfloat16, tag="big", name="tp")
                    for k in range(8):
                        nc.tensor.matmul(
                            tp[:, 128 * k : 128 * (k + 1)],
                            xb[:, ko + k, 128 * fs : 128 * (fs + 1)],
                            id128b_t[:],
                            is_transpose=True,
                        )
                    # all transpose drains on DVE: ACT is co-critical (gelus +
                    # table loads + ybuf drains) while DVE has slack
                    if ndr[0] % 8 < 7:
                        nc.vector.tensor_copy(xtb[:, fs, :], tp[:])
                    else:
                        nc.scalar.copy(xtb[:, fs, :], tp[:])
                    ndr[0] += 1
                xtbs[b] = xtb

            def emit_scores(q, hf):
                # one strictly-sequential accumulation group per (bank, hf):
                # all 8 blocks x 2 fs accumulate via zero-padded stationaries
                if hf == 0:
                    zaps[q] = ps_z.tile([64, 512], dt.float32, tag="zq", name="zaps")
                    zbrp[q] = ps_r.tile([64, 32], dt.float32, tag="rq", name="zbrp")
                for b8 in range(8):
                    xtb = xtbs[8 * q + b8]
                    for fs in range(2):
                        first = b8 == 0 and fs == 0
                        last = b8 == 7 and fs == 1
                        # skip_group_check: the sim's zero-region tracker
                        # mis-addresses partition-offset outputs; the two hf
                        # groups per bank are strictly sequential by design.
                        nc.tensor.matmul(
                            zaps[q][32 * hf : 32 * (hf + 1), :],
                            aebA_t[:, b8, fs, :],
                            xtb[:, fs, 512 * hf : 512 * (hf + 1)],
                            start=first,
                            stop=last,
                            skip_group_check=True,
                        )
                        nc.tensor.matmul(
                            zbrp[q][32 * hf : 32 * (hf + 1), :],
                            aebB_t[:, b8, fs, :],
                            xtb[:, fs, 512 * hf : 512 * (hf + 1)].rearrange(
                                "p (g j) -> p g j", j=16
                            )[:, :, 0:1],
                            start=first,
                            stop=last,
                            skip_group_check=True,
                        )
                if hf == 1:
                    for b8 in range(8):
                        xtbs.pop(8 * q + b8)

            def emit_C(q):
                # softmax epilogue on [64, 512]: rows (hf, b8, h), cols tokens
                zbr_sb = mpool.tile([64, 32], dt.float32, tag=f"zbr{q}", name="zbr_sb")
                nc.scalar.copy(zbr_sb[:], zbrp[q][:])
                zs = epool.tile([64, 512], dt.float32, tag="epsA", name="zs")
                nc.vector.tensor_add(
                    zs[:].rearrange("p (g j) -> p g j", j=16),
                    zaps[q][:].rearrange("p (g j) -> p g j", j=16),
                    zbr_sb[:].unsqueeze(2).broadcast_to([64, 32, 16]),
                )
                g1 = epool.tile([64, 512], dt.float32, tag="epsB", name="g1")
                nc.scalar.activation(g1[:], zs[:], AF.Gelu)
                sv = epool.tile([64, 512], dt.float32, tag="epsA", name="sv")
                nc.scalar.activation(sv[:], g1[:], AF.Gelu)
                e = epool.tile([64, 512], dt.float32, tag="epsB", name="e")
                nc.scalar.activation(e[:], sv[:], AF.Exp)
                den = mpool.tile([64, 32], dt.float32, tag=f"den{q}", name="den")
                nc.vector.reduce_sum(
                    den[:].unsqueeze(2),
                    e[:].rearrange("p (g j) -> p g j", j=16),
                    axis=mybir.AxisListType.X,
                )
                rec = mpool.tile([64, 32], dt.float32, tag=f"rec{q}", name="rec")
                nc.vector.reciprocal(rec[:], den[:])
                att = attpool.tile([64, 512], dt.float32r, tag="att", name="att")
                nc.vector.tensor_mul(
                    att[:].rearrange("p (g j) -> p g j", j=16),
                    e[:].rearrange("p (g j) -> p g j", j=16),
                    rec[:].unsqueeze(2).broadcast_to([64, 32, 16]),
                )
                att_sb[q] = att

            def emit_attT(q):
                # one transpose per 128-token window: [64, 128] -> [128, 64]
                a = ps_r.tile([128, 4, 64], dt.float32r, tag="rq", name="atp")
                for w in range(4):
                    nc.tensor.transpose(
                        a[:, w, :],
                        att_sb[q][:, 128 * w : 128 * (w + 1)],
                        id128_t[0:64, 0:64],
                    )
                atp[q] = a

            ybps = [None, None]

            def emit_D(c, ybuf):
                # stage-1 for blocks 4c..4c+4: S_att build + x-stationary matmul
                q = c // 2
                sabs = []
                # hoist the 4 S_att builds so the DVE stays ahead of the PE
                for bl in range(4):
                    b8 = (4 * c + bl) % 8
                    sab = sapool.tile([128, 8, 32], dt.bfloat16, tag="sab", name="sab")
                    # sab[p, (hf,w), (h,g)] = atp[q][p, w, 32*hf+4*b8+h] * mask[p, g]
                    for hf in range(2):
                        nc.vector.tensor_mul(
                            sab[:, 4 * hf : 4 * (hf + 1), :].rearrange(
                                "p w (hh g) -> p w hh g", hh=4
                            ),
                            f32(atp[q][:])[:, :, 32 * hf + 4 * b8 : 32 * hf + 4 * b8 + 4]
                            .unsqueeze(3)
                            .broadcast_to([128, 4, 4, 8]),
                            mm_t[:]
                            .rearrange("p (hh g) -> p hh g", g=8)
                            .unsqueeze(1)
                            .broadcast_to([128, 4, 4, 8]),
                        )
                    sabs.append(sab)
                for bl in range(4):
                    b = 4 * c + bl
                    sab = sabs[bl]
                    xb = xbs[b // 2]
                    ko = 8 * (b % 2)
                    for k in range(8):
                        K = b * 8 + k
                        kq = K % 16
                        if kq == 0:
                            # alternate accumulation banks into ps_big while it
                            # is idle (after B, before E0) to avoid back-to-back
                            # slot-reuse stalls on the drain latency
                            if c < 2 and (K // 16) % 2 == 1:
                                pl, tg = ps_big, "big"
                            else:
                                pl, tg = ps_s1, "s1"
                            ybps[0] = pl.tile([128, 512], dt.float32, tag=tg, name="yb0")
                            ybps[1] = pl.tile([128, 512], dt.float32, tag=tg, name="yb1")
                        for fs in range(2):
                            nc.tensor.matmul(
                                ybps[fs][:, 32 * kq : 32 * kq + 32],
                                xb[:, ko + k, 128 * fs : 128 * (fs + 1)],
                                sab[:, k, :],
                                start=(kq == 0),
                                stop=(kq == 15),
                            )
                        if kq == 15:
                            qq = (K % 32) // 16
                            co = 1024 * c + 512 * qq
                            nc.scalar.copy(ybuf[:, 0, co : co + 512], ybps[0][:])
                            nc.vector.tensor_copy(ybuf[:, 1, co : co + 512], ybps[1][:])

            def emit_E2(P, ybig):
                # stage-2 over a 2-chunk pair: N=512 moving streams, one
                # weight load per (h, ds, fs) -> x1T cols 512P:512(P+1)
                yb = ybig[:, :, 2048 * P : 2048 * (P + 1)]
                for ds in range(2):
                    ghs = []
                    for h in range(H):
                        o2 = ps_big.tile([128, 512], dt.float32, tag="big", name="o2")
                        for fs in range(2):
                            nc.tensor.matmul(
                                o2[:],
                                w0_t[:, h, fs, 128 * ds : 128 * (ds + 1)],
                                yb[:, fs, :].rearrange(
                                    "p (K hh g) -> p K hh g", hh=4, g=8
                                )[:, :, h, :],
                                start=(fs == 0),
                                stop=(fs == 1),
                            )
                        gh = ghpool.tile([128, 512], dt.bfloat16, tag="gh", name="gh")
                        nc.scalar.activation(gh[:], o2[:], AF.Gelu)
                        ghs.append(gh)
                    ad1 = adpool.tile([128, 512], dt.bfloat16, tag="ad", name="ad1")
                    nc.vector.tensor_add(ad1[:], ghs[0][:], ghs[1][:])
                    ad2 = adpool.tile([128, 512], dt.bfloat16, tag="ad", name="ad2")
                    nc.vector.tensor_add(ad2[:], ghs[2][:], ghs[3][:])
                    nc.vector.tensor_add(
                        x1T[:, ds, 512 * P : 512 * (P + 1)], ad1[:], ad2[:]
                    )

            def emit_x1n():
                # layer-1 prep: transpose x1T columns to token-major x1n;
                # 4 transposes share one PSUM bank -> 1 drain copy each
                for jp in range(4):
                    trp = ps_r.tile([128, 4, 128], dt.bfloat16, tag="rq", name="trp")
                    for m in range(4):
                        j, ds = 2 * jp + m // 2, m % 2
                        nc.tensor.transpose(
                            trp[:, m, :],
                            x1T[:, ds, 128 * j : 128 * (j + 1)],
                            id128b_t[:],
                        )
                    nc.vector.tensor_copy(
                        x1n[:, 2 * jp : 2 * jp + 2, :].rearrange(
                            "p j (ds d) -> p (j ds) d", ds=2
                        ),
                        trp[:],
                    )

            L1z = {}

            def emit_L1half(h):
                # layer-1 scores + softmax epilogue for token half h
                # (qt 2h, 2h+1): ACT/DVE chain overlaps D2/D3 PE work.
                # Slicing ae1A_t[..., 8h:8h+8] reuses the padded variants.
                z1a_h = ps_z.tile([8, 256], dt.float32, tag="zq", name="z1a_h")
                zbr1_h = ps_z.tile([8, 16], dt.float32, tag="zq", name="zbr1_h")
                for qt in (2 * h, 2 * h + 1):
                    for ds in range(2):
                        st = qt == 2 * h and ds == 0
                        sp = qt == 2 * h + 1 and ds == 1
                        nc.tensor.matmul(
                            z1a_h[:],
                            ae1A_t[:, qt, ds, 8 * h : 8 * h + 8],
                            x1T[:, ds, 256 * qt : 256 * (qt + 1)],
                            start=st,
                            stop=sp,
                        )
                        nc.tensor.matmul(
                            zbr1_h[:],
                            ae1B_t[:, qt, ds, 8 * h : 8 * h + 8],
                            x1T[:, ds, 256 * qt : 256 * (qt + 1)].rearrange(
                                "p (g j) -> p g j", j=16
                            )[:, :, 0:1],
                            start=st,
                            stop=sp,
                        )
                zbr1_sb = mpool.tile([8, 16], dt.float32, tag=f"zbr1s{h}", name="zbr1_sb")
                nc.scalar.copy(zbr1_sb[:], zbr1_h[:])
                zs1 = epool.tile([8, 256], dt.float32, tag="epsA", name="zs1")
                nc.vector.tensor_add(
                    zs1[:].rearrange("p (g j) -> p g j", j=16),
                    z1a_h[:].rearrange("p (g j) -> p g j", j=16),
                    zbr1_sb[:].unsqueeze(2).broadcast_to([8, 16, 16]),
                )
                g11 = epool.tile([8, 256], dt.float32, tag="epsB", name="g11")
                nc.scalar.activation(g11[:], zs1[:], AF.Gelu)
                s1t = epool.tile([8, 256], dt.float32, tag="epsA", name="s1t")
                nc.scalar.activation(s1t[:], g11[:], AF.Gelu)
                e1 = epool.tile([8, 256], dt.float32, tag="epsB", name="e1")
                nc.scalar.activation(e1[:], s1t[:], AF.Exp)
                den1 = mpool.tile([8, 16], dt.float32, tag=f"den1{h}", name="den1")
                nc.vector.reduce_sum(
                    den1[:].unsqueeze(2),
                    e1[:].rearrange("p (g j) -> p g j", j=16),
                    axis=mybir.AxisListType.X,
                )
                rec1 = mpool.tile([8, 16], dt.float32, tag=f"rec1{h}", name="rec1")
                nc.vector.reciprocal(rec1[:], den1[:])
                att1_h = attpool.tile(
                    [8, 256], dt.float32r, tag=f"att1{h}", bufs=1, name="att1_h"
                )
                nc.vector.tensor_mul(
                    att1_h[:].rearrange("p (g j) -> p g j", j=16),
                    e1[:].rearrange("p (g j) -> p g j", j=16),
                    rec1[:].unsqueeze(2).broadcast_to([8, 16, 16]),
                )
                L1z[h] = att1_h

            # ---- emission schedule ----
            for pp in range(4):
                emit_xload(pp)
            for b in range(8):
                emit_Bt(b)
                if b == 1:
                    for pp in range(4, 8):
                        emit_xload(pp)
                if rep == 0 and b == 2:
                    nc.scalar.dma_start(
                        out=w0_t[:], in_=w0_d.rearrange("h (fs p) d -> p h fs d", p=128)
                    )
                if rep == 0 and b == 4:
                    nc.scalar.dma_start(
                        out=w1_t[:], in_=w1_d.rearrange("h (fs p) d -> p h fs d", p=128)
                    )
            emit_scores(0, 0)
            emit_scores(0, 1)
            emit_C(0)
            for b in range(8, 16):
                emit_Bt(b)
            emit_scores(1, 0)
            emit_scores(1, 1)
            emit_attT(0)
            ybig = ybpool.tile([128, 2, 4096], dt.bfloat16, tag="ybuf", name="ybig")
            emit_D(0, ybig)
            emit_D(1, ybig)
            emit_C(1)
            emit_attT(1)
            emit_E2(0, ybig)
            emit_L1half(0)
            emit_D(2, ybig)
            emit_D(3, ybig)
            emit_E2(1, ybig)
            emit_L1half(1)

            # ============ LAYER 1 ============
            # x1n transposes first (PE), then per-half att transposes + S_att
            emit_x1n()
            sab1 = {}
            for h in range(2):
                att1_h = L1z[h]
                atp1 = ps_r.tile([128, 2, 8], dt.float32r, tag="rq", name="atp1")
                for w in range(2):
                    nc.tensor.transpose(
                        atp1[:, w, :],
                        att1_h[:, 128 * w : 128 * (w + 1)],
                        id128_t[0:8, 0:8],
                    )
                for w in range(2):
                    sb = sapool.tile([128, 2, 32], dt.bfloat16, tag="sab1", name="sab1")
                    nc.vector.tensor_mul(
                        sb[:].rearrange("p qt (hh g) -> p qt hh g", g=8),
                        f32(atp1[:, w, :])
                        .rearrange("p (qt hh) -> p qt hh", hh=4)
                        .unsqueeze(3)
                        .broadcast_to([128, 2, 4, 8]),
                        mm_t[:]
                        .rearrange("p (hh g) -> p hh g", g=8)
                        .unsqueeze(1)
                        .broadcast_to([128, 2, 4, 8]),
                    )
                    sab1[(h, w)] = sb
            # stage-1: single accumulation bank [128, 2, 256]
            y1p = ps_s1.tile([128, 2, 256], dt.float32, tag="s1", name="y1p")
            for k1 in range(8):
                qt, w = k1 // 2, k1 % 2
                for ds in range(2):
                    nc.tensor.matmul(
                        y1p[:, ds, 32 * k1 : 32 * k1 + 32],
                        x1n[:, k1, 128 * ds : 128 * (ds + 1)],
                        sab1[(qt // 2, w)][:, qt % 2, :],
                        start=(k1 == 0 and ds == 0),
                        stop=(k1 == 7 and ds == 1),
                    )
            y1b = mpool.tile([128, 2, 256], dt.bfloat16, tag="y1b", name="y1b")
            nc.vector.tensor_copy(y1b[:], y1p[:])

            # stage-2 + final output
            out_sb = mpool.tile([64, 256], dt.float32, tag="out_sb", name="out_sb")
            for d2s in range(2):
                ghs1 = []
                for hp in range(2):
                    o21 = ps_big.tile([128, 2, 64], dt.float32, tag="big", name="o21")
                    for hh in range(2):
                        h = 2 * hp + hh
                        for ds in range(2):
                            nc.tensor.matmul(
                                o21[:, hh, :],
                                w1_t[:, h, ds, 128 * d2s : 128 * (d2s + 1)],
                                y1b[:, ds, :].rearrange(
                                    "p (j hh g) -> p j hh g", hh=4, g=8
                                )[:, :, h, :],
                                start=(ds == 0),
                                stop=(ds == 1),
                            )
                    gh = ghpool.tile([128, 2, 64], dt.float32, tag="gh1", name="gh1")
                    nc.scalar.activation(gh[:], o21[:], AF.Gelu)
                    ghs1.append(gh)
                ad1 = adpool.tile([128, 64], dt.float32, tag="ad1", name="ad1")
                nc.vector.tensor_add(ad1[:], ghs1[0][:, 0, :], ghs1[0][:, 1, :])
                ad2 = adpool.tile([128, 64], dt.float32, tag="ad1", name="ad2")
                nc.vector.tensor_add(ad2[:], ghs1[1][:, 0, :], ghs1[1][:, 1, :])
                u = mpool.tile([128, 64], dt.float32, tag=f"u{d2s}", name="u")
                nc.vector.tensor_add(u[:], ad1[:], ad2[:])
                uT = mpool.tile([128, 64], dt.float32r, tag=f"uT{d2s}", name="uT")
                nc.vector.tensor_scalar_mul(uT[:], u[:], 0.25)
                otp = ps_big.tile([64, 128], dt.float32r, tag="big", name="otp")
                nc.tensor.transpose(otp[:], uT[:], id128_t[:])
                nc.vector.tensor_copy(out_sb[:, 128 * d2s : 128 * (d2s + 1)], f32(otp[:]))
            # single output store: each DMA carries ~2us completion latency
            nc.scalar.dma_start(out=out_d[:, :], in_=out_sb[:])
    nc.compile()
    return nc


def _prep_weights(W0, A0, W1, A1):
    import ml_dtypes

    def effs(W, A):
        # a_eff[h] = W[h] @ A[h,:256,0]; b_eff[h] = W[h] @ A[h,256:,0]  -> [F, H]
        a = np.einsum("hfd,hd->hf", W.astype(np.float64), A[:, :256, 0].astype(np.float64))
        b = np.einsum("hfd,hd->hf", W.astype(np.float64), A[:, 256:, 0].astype(np.float64))
        return a.T.astype(np.float32), b.T.astype(np.float32)

    a0, b0 = effs(W0, A0)  # [256, 4] each
    a1, b1 = effs(W1, A1)
    a1, b1 = 0.25 * a1, 0.25 * b1

    import ml_dtypes as md

    def pack8(a, b):
        # [2, 128, 8]: cols 0:4 = a heads, 4:8 = b heads
        out = np.concatenate([a, b], axis=1)  # [256, 8]
        return np.ascontiguousarray(out.reshape(2, 128, 8).astype(md.bfloat16))

    t = np.arange(128)
    c = np.arange(32)
    mmask = ((c[None, :] % 8) == (t[:, None] // 16)).astype(np.float32)
    return {
        "w0": np.ascontiguousarray(W0.astype(md.bfloat16)),
        "w1": np.ascontiguousarray((0.25 * W1).astype(md.bfloat16)),
        "aeb": pack8(a0, b0),
        "ae1": pack8(a1, b1),
        "mmask": mmask,
        "id128": np.eye(128, dtype=np.float32),
        "id128b": np.eye(128, dtype=md.bfloat16),
    }


def _prep_x(x):
    import ml_dtypes

    return np.ascontiguousarray(np.asarray(x, np.float32).astype(ml_dtypes.bfloat16))


def kernel(x, W0, A0, W1, A1):
    x = np.asarray(x, dtype=np.float32)
    W0 = np.asarray(W0, dtype=np.float32)
    A0 = np.asarray(A0, dtype=np.float32)
    W1 = np.asarray(W1, dtype=np.float32)
    A1 = np.asarray(A1, dtype=np.float32)

    if "nc" not in _CACHE:
        _CACHE["nc"] = build_program()
    nc = _CACHE["nc"]

    wmap = _prep_weights(W0, A0, W1, A1)
    xs = _prep_x(x).reshape(NCORES, T0, F)
    in_maps = [dict(wmap, x=np.ascontiguousarray(xs[i])) for i in range(NCORES)]
    res = run_bass_kernel_spmd(
        nc, in_maps, core_ids=list(range(NCORES)), trace=TRACE
    )
    _CACHE["last_result"] = res
    out = np.concatenate([res.results[i]["out"] for i in range(NCORES)], axis=0)
    return out
